# revision 1
# baseline (speedup 1.0000x reference)
"""Trainium2 Bass kernel: GNN message passing (iterative Laplacian diffusion).

Problem: u0 = F@Ws + bs + elu(F@W1 + b1)@W2 + b2;  16x: u <- u - 0.1*(L@u)
  F: [16384, 512] fp32, L: [16384, 16384] fp32, output u: [16384, 16] fp32.

Strategy (8 NeuronCores, row-parallel SpMM):
  - Shard L row-wise (2048 rows/core).  The 16 diffusion steps re-read the
    whole L shard each step => memory-bound on HBM.  We compress L to
    fp8e4 (scaled by sigma2 * 2^17 so entries sit in e4m3's sweet spot),
    halving-again traffic vs bf16: 32 MiB/step/core.  The carried state u
    stays fp32; only the matmul operands are fp8, and each step's update is
    ~1e-3 of u, so quantization error lands at ~1e-4 relative overall.
  - Host pre-transposes the shard (lhsT layout) and pre-permutes its
    128-row k-chunks so chunk j of core i is logical chunk (16*i+j) % 128:
    every core starts each step with its OWN 16 chunks, whose u-tiles come
    from the local AXPY (no collective wait), hiding the per-step AllGather
    (~5us) behind the first 16/128 of the matmul work.  The other 112
    chunks read u from a doubled (wrap-free) SBUF buffer at a per-core
    register offset (rot input) — same SPMD instruction stream on all cores.
  - PE mapping: lhsT = L^T block [128,128] fp8 (FWL 4B/cycle weight load),
    rhs = u chunk [128,16] fp8, accumulate 128 k-chunks into one PSUM bank
    holding all 16 row-tiles as 16-column slices ([128, 256] fp32).
"""

import numpy as np
import ml_dtypes
from dataclasses import dataclass

from concourse import bass, bacc, tile
import concourse.mybir as mybir
from concourse.bass_utils import run_bass_kernel_spmd

F32 = mybir.dt.float32
FP8 = mybir.dt.float8e4
U32 = mybir.dt.uint32
P = 128  # partitions


@dataclass(frozen=True)
class Cfg:
    C: int = 8          # cores
    N: int = 16384      # nodes
    IN_F: int = 512     # input features
    HID: int = 256      # hidden dim
    OUT: int = 16       # output features
    STEPS: int = 16
    SIGMA2: float = 0.1
    SCALE: float = 2.0 ** 17   # fp8 encoding scale for (sigma2*L)

    @property
    def R(self):   # rows per core
        return self.N // self.C

    @property
    def MT(self):  # row-tiles (= own k-chunks) per core
        return self.R // P

    @property
    def KC(self):  # total k-chunks
        return self.N // P

    @property
    def NTILE(self):  # phase-1 rhs tile width
        return min(512, self.R)


def build_program(cfg: Cfg):
    C, R, MT, KC, OUT, STEPS = cfg.C, cfg.R, cfg.MT, cfg.KC, cfg.OUT, cfg.STEPS
    IN_F, HID = cfg.IN_F, cfg.HID
    KI = IN_F // P   # 4 input-feature k-chunks
    KH = HID // P    # 2 hidden k-chunks
    NT = cfg.NTILE
    AXPY_C = -1.0 / cfg.SCALE

    nc = bacc.Bacc("TRN2", target_bir_lowering=False, debug=False,
                   enable_asserts=False, num_devices=C)

    lapT = nc.dram_tensor("lapT", [cfg.N, R], FP8, kind="ExternalInput")
    featT = nc.dram_tensor("featT", [P, KI * R], F32, kind="ExternalInput")
    w1_t = nc.dram_tensor("w1_t", [P, KI * KH * P], F32, kind="ExternalInput")
    ws_t = nc.dram_tensor("ws_t", [P, KI * OUT], F32, kind="ExternalInput")
    w2_t = nc.dram_tensor("w2_t", [P, KH * OUT], F32, kind="ExternalInput")
    b1_t = nc.dram_tensor("b1_t", [P, KH], F32, kind="ExternalInput")
    biasT = nc.dram_tensor("biasT", [OUT, 1], F32, kind="ExternalInput")
    ident = nc.dram_tensor("ident", [OUT, OUT], F32, kind="ExternalInput")
    rot = nc.dram_tensor("rot", [1, 1], U32, kind="ExternalInput")
    out_u = nc.dram_tensor("out_u", [R, OUT], F32, kind="ExternalOutput")

    AF = mybir.ActivationFunctionType
    ALU = mybir.AluOpType

    with tile.TileContext(nc) as tc:
        with (
            tc.tile_pool(name="slabp", bufs=12) as slabp,
            tc.tile_pool(name="resp", bufs=1) as resp,
            tc.tile_pool(name="upool", bufs=2) as upool,
            tc.tile_pool(name="urotp", bufs=2) as urotp,
            tc.tile_pool(name="ownp", bufs=2) as ownp,
            tc.tile_pool(name="u32p", bufs=2) as u32p,
            tc.tile_pool(name="constp", bufs=1) as constp,
            tc.tile_pool(name="zpsum", bufs=2, space="PSUM") as zpsum,
            tc.tile_pool(name="psTp", bufs=2, space="PSUM") as psTp,
            tc.tile_pool(name="dramp", bufs=2, space="DRAM") as dramp,
        ):
            # per-core rotation offset -> DVE register: used by one dynamic
            # tensor_copy per step that rotates the gathered u into chunk
            # order; all matmul APs stay static (keeps FWL weight loads).
            rot_s = constp.tile([1, 1], U32, name="rot_s")
            nc.sync.dma_start(rot_s[:], rot[:])
            reg = nc.alloc_registers("rotreg", engines=[mybir.EngineType.DVE])
            nc.regs_load(reg, rot_s[0:1, 0:1])
            sv_base = nc.snap(reg, donate=True, min_val=0,
                              max_val=(C - 1) * MT * OUT)

            u32_cur = u32p.tile([OUT, R], F32, name="u32_init", tag="u32")
            ident_s = constp.tile([OUT, OUT], F32, name="ident_s")
            nc.sync.dma_start(ident_s[:], ident[:])
            biasT_s = constp.tile([OUT, 1], F32, name="biasT_s")
            nc.sync.dma_start(biasT_s[:], biasT[:])

            # ---------------- MLP head: u0 = F@Ws + bs + elu(F@W1+b1)@W2 + b2
            with (
                tc.tile_pool(name="mlpp", bufs=1) as mlpp,
                tc.tile_pool(name="tmpp", bufs=2) as tmpp,
            ):
                featT_s = mlpp.tile([P, KI * R], F32, name="featT_s")
                nc.sync.dma_start(featT_s[:], featT[:])
                w1_s = mlpp.tile([P, KI * KH * P], F32, name="w1_s")
                nc.sync.dma_start(w1_s[:], w1_t[:])
                ws_s = mlpp.tile([P, KI * OUT], F32, name="ws_s")
                nc.sync.dma_start(ws_s[:], ws_t[:])
                w2_s = mlpp.tile([P, KH * OUT], F32, name="w2_s")
                nc.sync.dma_start(w2_s[:], w2_t[:])
                b1_s = mlpp.tile([P, KH], F32, name="b1_s")
                nc.sync.dma_start(b1_s[:], b1_t[:])

                # phase 1: hT[kappa, mt*R + n] = elu(F@W1 + b1)^T
                hT = mlpp.tile([P, KH * R], F32, name="hT")
                for mt in range(KH):
                    for nt in range(R // NT):
                        ps = zpsum.tile([P, NT], F32, name="ps1", tag="ps1")
                        for k in range(KI):
                            nc.tensor.matmul(
                                ps[:],
                                w1_s[:, (k * KH + mt) * P:(k * KH + mt + 1) * P],
                                featT_s[:, k * R + nt * NT: k * R + (nt + 1) * NT],
                                start=(k == 0), stop=(k == KI - 1),
                            )
                        b1_ap = b1_s[:, mt:mt + 1]
                        t_relu = tmpp.tile([P, NT], F32, name="t_relu", tag="t_relu")
                        nc.scalar.activation(t_relu[:], ps[:], AF.Relu, bias=b1_ap)
                        t_exp = tmpp.tile([P, NT], F32, name="t_exp", tag="t_exp")
                        nc.scalar.activation(t_exp[:], ps[:], AF.Exp, bias=b1_ap)
                        t_min = tmpp.tile([P, NT], F32, name="t_min", tag="t_min")
                        nc.vector.tensor_scalar(t_min[:], t_exp[:], 1.0, -1.0,
                                                ALU.min, ALU.add)
                        nc.vector.tensor_tensor(
                            hT[:, mt * R + nt * NT: mt * R + (nt + 1) * NT],
                            t_min[:], t_relu[:], ALU.add)

                # phase 2 (transposed): u0T[j, r] built per 512-row tile
                for nt in range(R // NT):
                    ps2 = zpsum.tile([OUT, NT], F32, name="ps2",
                                      tag=f"zps{nt}", bufs=1)
                    for k in range(KI):
                        nc.tensor.matmul(
                            ps2[:],
                            ws_s[:, k * OUT:(k + 1) * OUT],
                            featT_s[:, k * R + nt * NT: k * R + (nt + 1) * NT],
                            start=(k == 0), stop=False,
                        )
                    for k2 in range(KH):
                        nc.tensor.matmul(
                            ps2[:],
                            w2_s[:, k2 * OUT:(k2 + 1) * OUT],
                            hT[:, k2 * R + nt * NT: k2 * R + (nt + 1) * NT],
                            start=False, stop=(k2 == KH - 1),
                        )
                    nc.vector.tensor_scalar_add(
                        u32_cur[:, nt * NT:(nt + 1) * NT], ps2[:],
                        biasT_s[:, 0:1])

            # ---------------- diffusion loop (DoubleRow z^T mapping)
            # u carried transposed: u32T [OUT, R].  Weights = u chunk-pairs
            # [128, 2, 16] (static slices of own/u_rot), rhs = slab-pairs
            # [128, 2, 512] (fp8 DoubleRow: 2 fp8/cell, 2 mul/cycle), out =
            # z^T [16, 512] in 4 PSUM banks, accumulated over 64 pairs.
            KP = KC // 2          # superchunk (chunk-pair) count
            MP = MT // 2          # own superchunks
            NT2 = min(512, R)
            NNT = R // NT2        # n-tiles of z^T

            def transpose_to_own(u32T_tile, out_fp8):
                """u32T [OUT, R] -> row-partition fp8 [P, MT*OUT] via PE."""
                for b in range(MT):
                    pt = psTp.tile([P, OUT], F32, name="psT", tag="psT")
                    nc.tensor.transpose(
                        pt[:], u32T_tile[:, b * P:(b + 1) * P], ident_s[:])
                    nc.scalar.activation(
                        out_fp8[:, b * OUT:(b + 1) * OUT], pt[:], AF.Copy)

            def cast_and_gather(u32T_tile):
                own = ownp.tile([P, MT * OUT], FP8, name="own", tag="own")
                transpose_to_own(u32T_tile, own)
                agin = dramp.tile([P, MT * OUT], FP8, name="agin", tag="agin")
                agout = dramp.tile([C * P, MT * OUT], FP8, name="agout",
                                   tag="agout", addr_space="Shared")
                nc.scalar.dma_start(agin[:], own[:])
                nc.gpsimd.collective_compute(
                    "AllGather", ALU.bypass,
                    replica_groups=[list(range(C))],
                    ins=[agin.opt()], outs=[agout.opt()],
                )
                ub = upool.tile([P, 2 * KC * OUT], FP8, name="ub", tag="ub")
                src = agout[:].rearrange("(r k) m -> k r m", k=P)
                for h in range(2):
                    dst = ub[:, h * KC * OUT:(h + 1) * KC * OUT]
                    nc.scalar.dma_start(
                        dst.rearrange("k (r m) -> k r m", r=C), src)
                u_rot = urotp.tile([P, KC * OUT], FP8, name="u_rot", tag="u_rot")
                nc.vector.tensor_copy(u_rot[:], ub[:, bass.ds(sv_base, KC * OUT)])
                return own, u_rot

            RESP = min(10, max(0, KP - 4))
            res_slabs = []
            for jp in range(RESP):
                rs = resp.tile([P, 2 * R], FP8, name=f"res{jp}", tag=f"res{jp}")
                nc.sync.dma_start(
                    rs[:].rearrange("k (s m) -> k s m", s=2),
                    lapT[jp * 2 * P:(jp + 1) * 2 * P, :].rearrange(
                        "(s k) m -> k s m", k=P))
                res_slabs.append(rs)

            own_cur, ub_cur = cast_and_gather(u32_cur)
            DR = mybir.MatmulPerfMode.DoubleRow

            for t in range(STEPS):
                zps = [zpsum.tile([OUT, NT2], F32, name=f"zps{nt}",
                                  tag=f"zps{nt}", bufs=1)
                       for nt in range(NNT)]
                for jp in range(KP):
                    if jp < RESP:
                        slab = res_slabs[jp]
                    else:
                        slab = slabp.tile([P, 2 * R], FP8, name="slab",
                                          tag="slab")
                        nc.sync.dma_start(
                            slab[:].rearrange("k (s m) -> k s m", s=2),
                            lapT[jp * 2 * P:(jp + 1) * 2 * P, :].rearrange(
                                "(s k) m -> k s m", k=P))
                    if jp < MP:
                        lh = own_cur[:, 2 * OUT * jp:2 * OUT * (jp + 1)]
                    else:
                        lh = ub_cur[:, 2 * OUT * jp:2 * OUT * (jp + 1)]
                    lh3 = lh.rearrange("k (s c) -> k s c", s=2)
                    s3 = slab[:].rearrange("k (s m) -> k s m", s=2)
                    for nt in range(NNT):
                        nc.tensor.matmul(
                            zps[nt][:], lh3,
                            s3[:, :, nt * NT2:(nt + 1) * NT2],
                            start=(jp == 0), stop=(jp == KP - 1),
                            perf_mode=DR,
                        )
                u32_new = u32p.tile([OUT, R], F32, name="u32", tag="u32")
                for nt in range(NNT):
                    nc.vector.scalar_tensor_tensor(
                        u32_new[:, nt * NT2:(nt + 1) * NT2], zps[nt][:],
                        AXPY_C, u32_cur[:, nt * NT2:(nt + 1) * NT2],
                        ALU.mult, ALU.add)
                u32_cur = u32_new
                if t < STEPS - 1:
                    own_cur, ub_cur = cast_and_gather(u32_cur)
                else:
                    u32f = constp.tile([P, MT * OUT], F32, name="u32f")
                    for b in range(MT):
                        pt = psTp.tile([P, OUT], F32, name="psT", tag="psT")
                        nc.tensor.transpose(
                            pt[:], u32_cur[:, b * P:(b + 1) * P], ident_s[:])
                        nc.scalar.activation(
                            u32f[:, b * OUT:(b + 1) * OUT], pt[:], AF.Copy)
                    nc.gpsimd.dma_start(
                        out_u[:].rearrange("(m k) j -> k m j", k=P),
                        u32f[:].rearrange("k (m j) -> k m j", j=OUT))

    nc.compile()
    return nc


def host_prep(cfg: Cfg, features, laplacian, W1, b1, W2, b2, Ws, bs):
    C, R, MT, KC, OUT = cfg.C, cfg.R, cfg.MT, cfg.KC, cfg.OUT
    KI, KH = cfg.IN_F // P, cfg.HID // P
    F = np.ascontiguousarray(np.asarray(features, np.float32))
    L = np.asarray(laplacian, np.float32)
    W1 = np.asarray(W1, np.float32)
    b1 = np.asarray(b1, np.float32)
    W2 = np.asarray(W2, np.float32)
    b2 = np.asarray(b2, np.float32)
    Ws = np.asarray(Ws, np.float32)
    bs = np.asarray(bs, np.float32)

    Lq = (L * np.float32(cfg.SIGMA2 * cfg.SCALE)).astype(ml_dtypes.float8_e4m3)

    w1_t = np.ascontiguousarray(
        W1.reshape(KI, P, KH, P).transpose(1, 0, 2, 3).reshape(P, KI * KH * P))
    ws_t = np.ascontiguousarray(
        Ws.reshape(KI, P, OUT).transpose(1, 0, 2).reshape(P, KI * OUT))
    w2_t = np.ascontiguousarray(
        W2.reshape(KH, P, OUT).transpose(1, 0, 2).reshape(P, KH * OUT))
    b1_t = np.ascontiguousarray(b1.reshape(KH, P).T)
    biasT = np.ascontiguousarray((bs + b2).astype(np.float32).reshape(-1, 1))
    ident = np.eye(len(bs), dtype=np.float32)

    in_maps = []
    for i in range(C):
        shard = Lq[i * R:(i + 1) * R, :]                   # [R, N]
        Ti = np.ascontiguousarray(shard.T)                 # [N, R] lhsT layout
        perm = [(MT * i + j) % KC for j in range(KC)]
        Ti_p = np.ascontiguousarray(
            Ti.reshape(KC, P, R)[perm].reshape(cfg.N, R))
        Fi = F[i * R:(i + 1) * R, :]
        featT_i = np.ascontiguousarray(
            Fi.T.reshape(KI, P, R).transpose(1, 0, 2).reshape(P, KI * R))
        in_maps.append({
            "lapT": Ti_p,
            "featT": featT_i,
            "w1_t": w1_t,
            "ws_t": ws_t,
            "w2_t": w2_t,
            "b1_t": b1_t,
            "biasT": biasT,
            "ident": ident,
            "rot": np.array([[i * MT * OUT]], np.uint32),
        })
    return in_maps


_NC_CACHE = {}


def _get_nc(cfg: Cfg):
    if cfg not in _NC_CACHE:
        _NC_CACHE[cfg] = build_program(cfg)
    return _NC_CACHE[cfg]


def _install_ntff_hook():
    """Recreate antenv.axon_hooks (absent in this image) so
    run_bass_kernel_spmd(trace=True) can NTFF-profile via libaxon_pjrt."""
    import sys
    import types
    import ctypes
    import contextlib

    if "antenv.axon_hooks" in sys.modules:
        return
    so_path = "/opt/axon/libaxon_pjrt.so"
    lib = ctypes.CDLL(so_path)
    if not hasattr(lib, "axon_start_nrt_profile"):
        return
    lib.axon_start_nrt_profile.argtypes = [
        ctypes.POINTER(ctypes.c_int64), ctypes.c_size_t]
    lib.axon_start_nrt_profile.restype = ctypes.c_int64
    lib.axon_stop_nrt_profile.argtypes = [ctypes.c_char_p]
    lib.axon_stop_nrt_profile.restype = ctypes.c_int64

    @contextlib.contextmanager
    def _hook(output_dir, device_ids):
        import jax
        jax.devices()
        if device_ids:
            ids = (ctypes.c_int64 * len(device_ids))(*device_ids)
            rc = lib.axon_start_nrt_profile(ids, len(device_ids))
        else:
            rc = lib.axon_start_nrt_profile(None, 0)
        if rc != 0:
            raise RuntimeError(f"axon_start_nrt_profile rc={rc}")
        try:
            yield
        finally:
            n = lib.axon_stop_nrt_profile(str(output_dir).encode())
            print(f"profile: {n} file(s) written to {output_dir}")

    mod = types.ModuleType("antenv.axon_hooks")
    mod.get_axon_ntff_profile_hook = lambda: _hook
    mod.set_axon_ntff_profile_hook = lambda h: None
    sys.modules["antenv.axon_hooks"] = mod


def run(inputs, cfg: Cfg = Cfg(), trace: bool = False):
    if trace:
        _install_ntff_hook()
    nc = _get_nc(cfg)
    in_maps = host_prep(cfg, **inputs)
    res = run_bass_kernel_spmd(nc, in_maps, core_ids=list(range(cfg.C)),
                               trace=trace)
    out = np.concatenate([res.results[i]["out_u"] for i in range(cfg.C)], axis=0)
    return out, res


def kernel(**inputs):
    out, _ = run(inputs)
    return out



# revision 10
# speedup vs baseline: 1.2847x; 1.2847x over previous
"""Trainium2 Bass kernel: GNN message passing (iterative Laplacian diffusion).

Problem: u0 = F@Ws + bs + elu(F@W1 + b1)@W2 + b2;  16x: u <- u - 0.1*(L@u)
  F: [16384, 512] fp32, L: [16384, 16384] fp32, output u: [16384, 16] fp32.

v2 strategy (8 NeuronCores, row-parallel SpMM):
  - L row-sharded (2048 rows/core), compressed to fp8e4 (scale sigma2*2^17).
    Each of the 16 steps re-streams the L shard => memory bound; RESP chunks
    (~19 MiB) stay resident in SBUF, the rest stream per step.
  - PE mapping: z^T[j, n] accumulated over 128 k-chunks.  Four col-tiled
    matmul groups (tile_position (0,32g)) run CONCURRENTLY, one per 512-node
    n-tile, each streaming its own L slab slice (4B/part/cycle aggregate,
    2x the DoubleRow rate).  Groups 0,1 share one PSUM bank via a single
    start/stop accumulation chain; groups 2,3 share another.
  - Two half-AllGathers per step, software-pipelined: groups 2,3 issue at
    3/5 the rate of groups 0,1, so half A of z finishes ~9us early; its
    AXPY/transpose/cast/AllGather chain runs while half B still matmuls.
    Each step processes own chunks first, then half-A foreigns, then
    half-B foreigns, so gathers land just in time and the PE never idles
    long enough to drop the HAM clock gate to the cold 1.2GHz state.
  - u is carried TRANSPOSED ([16, nodes] in 4 partition stripes 32g..32g+15
    of a [128,512] tile).  AXPY runs in that layout straight from PSUM;
    PE transposes produce the node-major fp8 copy used as stationary
    weights (and gathered).  Rotation trick: host permutes L chunks so all
    cores run one instruction stream; a DVE register-offset copy per half
    rotates the gathered u into processing order.
"""

import numpy as np
import ml_dtypes
from dataclasses import dataclass

from concourse import bass, bacc, tile
import concourse.mybir as mybir
from concourse.bass_utils import run_bass_kernel_spmd

F32 = mybir.dt.float32
BF16 = mybir.dt.bfloat16
FP8 = mybir.dt.float8e4
U32 = mybir.dt.uint32
P = 128  # partitions


@dataclass(frozen=True)
class Cfg:
    C: int = 8          # cores
    N: int = 16384      # nodes
    IN_F: int = 512     # input features
    HID: int = 256      # hidden dim
    OUT: int = 16       # output features
    STEPS: int = 16
    SIGMA2: float = 0.1
    SCALE: float = 2.0 ** 17   # fp8 encoding scale for (sigma2*L)
    RESP: int = 76      # resident L chunks (of KC) per core
    SLAB_BUFS: int = 3  # streamed half-slab double-buffering

    @property
    def R(self):   # rows per core
        return self.N // self.C

    @property
    def MT(self):  # 128-row blocks per core
        return self.R // P

    @property
    def KC(self):  # total k-chunks
        return self.N // P

    @property
    def NT(self):  # n-tile width (one col-tile group's slice)
        return 512


def build_program(cfg: Cfg):
    C, R, MT, KC, OUT, STEPS = cfg.C, cfg.R, cfg.MT, cfg.KC, cfg.OUT, cfg.STEPS
    IN_F, HID, NT = cfg.IN_F, cfg.HID, cfg.NT
    KI = IN_F // P   # 4 input-feature k-chunks
    KH = HID // P    # 2 hidden k-chunks
    NNT = R // NT    # 4 n-tiles = 4 col-tile groups
    RESP = cfg.RESP
    AXPY_C = -1.0 / cfg.SCALE
    HB = MT // 2          # node blocks per half (8)
    HCOLS = HB * OUT      # own fp8 cols per half (128)
    REG = C * HCOLS       # gathered region cols per half (1024)
    HR = R // 2           # slab columns per half (1024)

    assert RESP % 4 == 0 and RESP >= MT

    nc = bacc.Bacc("TRN2", target_bir_lowering=False, debug=False,
                   enable_asserts=False, num_devices=C)

    lapT = nc.dram_tensor("lapT", [cfg.N, R], FP8, kind="ExternalInput")
    featT = nc.dram_tensor("featT", [P, KI * R], BF16, kind="ExternalInput")
    w1_t = nc.dram_tensor("w1_t", [P, KI * KH * P], BF16, kind="ExternalInput")
    ws_t = nc.dram_tensor("ws_t", [P, KI * OUT], BF16, kind="ExternalInput")
    w2_t = nc.dram_tensor("w2_t", [P, KH * OUT], BF16, kind="ExternalInput")
    b1_t = nc.dram_tensor("b1_t", [P, KH], F32, kind="ExternalInput")
    biasT = nc.dram_tensor("biasT", [P, 1], F32, kind="ExternalInput")
    ident = nc.dram_tensor("ident", [P, OUT], F32, kind="ExternalInput")
    rot = nc.dram_tensor("rot", [1, 1], U32, kind="ExternalInput")
    out_u = nc.dram_tensor("out_u", [R, OUT], F32, kind="ExternalOutput")

    AF = mybir.ActivationFunctionType
    ALU = mybir.AluOpType

    # position p (processing order) -> stationary-weight slice
    # p<16: own block p; 16<=p<72: half-A foreign; p>=72: half-B foreign
    def wslice(p, own, u_rotA, u_rotB):
        if p < MT:
            return own[:, p * OUT:(p + 1) * OUT]
        if p < MT + 7 * HB:
            d = p - MT
            q, l = 1 + d // HB, d % HB
            off = q * HCOLS + l * OUT
            return u_rotA[:, off:off + OUT]
        d = p - (MT + 7 * HB)
        q, l = 1 + d // HB, d % HB
        off = q * HCOLS + l * OUT
        return u_rotB[:, off:off + OUT]

    with tile.TileContext(nc) as tc:
        with (
            tc.tile_pool(name="resp", bufs=1) as resp,
            tc.tile_pool(name="constp", bufs=1) as constp,
            tc.tile_pool(name="utp", bufs=2) as utp,
            tc.tile_pool(name="ownp", bufs=2) as ownp,
            tc.tile_pool(name="ubp", bufs=1) as ubp,
            tc.tile_pool(name="urotp", bufs=2) as urotp,
            tc.tile_pool(name="zps0_p", bufs=1, space="PSUM") as zps0_p,
            tc.tile_pool(name="zps1_p", bufs=1, space="PSUM") as zps1_p,
            tc.tile_pool(name="zps2_p", bufs=1, space="PSUM") as zps2_p,
            tc.tile_pool(name="zps3_p", bufs=1, space="PSUM") as zps3_p,
            tc.tile_pool(name="ptp", bufs=2, space="PSUM") as ptp,
            tc.tile_pool(name="dramp", bufs=2, space="DRAM") as dramp,
        ):
            # ---- rotation register (per-core offset into gathered halves)
            rot_s = constp.tile([1, 1], U32, name="rot_s")
            nc.sync.dma_start(rot_s[:], rot[:])
            reg = nc.alloc_registers("rotreg", engines=[mybir.EngineType.DVE])
            nc.regs_load(reg, rot_s[0:1, 0:1])
            sv = nc.snap(reg, donate=True, min_val=0, max_val=(C - 1) * HCOLS)

            ident_s = constp.tile([P, OUT], F32, name="ident_s")
            nc.sync.dma_start(ident_s[:], ident[:])
            biasT_s = constp.tile([P, 1], F32, name="biasT_s")
            nc.sync.dma_start(biasT_s[:], biasT[:])
            uout = constp.tile([P, MT * OUT], F32, name="uout")

            # ---- warm up the collectives path with a tiny dummy AllGather
            zsrc = constp.tile([1, P], FP8, name="zsrc")
            nc.vector.memset(zsrc[:], 0.0)
            dummy_i = dramp.tile([1, P], FP8, name="dummy_i", tag="dummy_i")
            dummy_o = dramp.tile([C, P], FP8, name="dummy_o", tag="dummy_o",
                                 addr_space="Shared")
            nc.scalar.dma_start(dummy_i[:], zsrc[:])
            nc.gpsimd.collective_compute(
                "AllGather", ALU.bypass, replica_groups=[list(range(C))],
                ins=[dummy_i.opt()], outs=[dummy_o.opt()])

            # ---- resident L chunks (positions 0..RESP-1), big DMAs
            res = resp.tile([P, RESP * R], FP8, name="res")
            CH = 16  # chunks per resident dma (4 MiB each)
            for c0 in range(0, RESP, CH):
                cw = min(CH, RESP - c0)
                nc.sync.dma_start(
                    res[:, c0 * R:(c0 + cw) * R].rearrange(
                        "k (q m) -> k q m", q=cw),
                    lapT[c0 * P:(c0 + cw) * P, :].rearrange(
                        "(q k) m -> k q m", k=P))

            # ---- PSUM tiles: one bank per col-tile group
            zps = [
                zps0_p.tile([P, NT], F32, name="zps0", tag="zps0"),
                zps1_p.tile([P, NT], F32, name="zps1", tag="zps1"),
                zps2_p.tile([P, NT], F32, name="zps2", tag="zps2"),
                zps3_p.tile([P, NT], F32, name="zps3", tag="zps3"),
            ]

            state = {"uT": None, "own": None, "urA": None, "urB": None}

            # ---------- tail compute: half h of step t (t==-1: init/bias-add)
            # axpy -> 8 transposes -> fp8 cast -> agin DMA -> AllGather trigger
            def tail_compute(t, h, uT_new, own_new):
                for g in (0, 1) if h == 0 else (2, 3):
                    lo = 32 * g
                    if t < 0:
                        nc.vector.tensor_scalar_add(
                            uT_new[lo:lo + OUT, :], zps[g][lo:lo + OUT, :],
                            biasT_s[lo:lo + OUT, 0:1])
                    else:
                        nc.vector.scalar_tensor_tensor(
                            uT_new[lo:lo + OUT, :], zps[g][lo:lo + OUT, :],
                            AXPY_C, state["uT"][lo:lo + OUT, :],
                            ALU.mult, ALU.add)
                dst = uout if t == STEPS - 1 else own_new
                for bi in range(HB):
                    b = h * HB + bi
                    g = b // NNT
                    pt = ptp.tile([P, OUT], F32, name="pt", tag="pt")
                    nc.tensor.matmul(
                        pt[:],
                        uT_new[32 * g:32 * g + OUT,
                               (b % NNT) * P:(b % NNT + 1) * P],
                        ident_s[32 * g:32 * g + OUT, :],
                        is_transpose=True, tile_position=(32 * g, 0),
                    )
                    nc.scalar.activation(
                        dst[:, b * OUT:(b + 1) * OUT], pt[:], AF.Copy)
                if t == STEPS - 1:
                    if h == 1:
                        nc.gpsimd.dma_start(
                            out_u[:].rearrange("(m k) j -> k m j", k=P),
                            uout[:].rearrange("k (m j) -> k m j", j=OUT))
                    return None
                agi = dramp.tile([P, HCOLS], FP8, name=f"agi{h}", tag=f"agi{h}")
                ago = dramp.tile([C * P, HCOLS], FP8, name=f"ago{h}",
                                 tag=f"ago{h}", addr_space="Shared")
                nc.scalar.dma_start(agi[:], own_new[:, h * HCOLS:(h + 1) * HCOLS])
                nc.gpsimd.collective_compute(
                    "AllGather", ALU.bypass, replica_groups=[list(range(C))],
                    ins=[agi.opt()], outs=[ago.opt()])
                return ago

            # ---------- tail recv: doubled rearrange-in + rotation copy
            def tail_recv(h, ago):
                ub = ubp.tile([P, 2 * REG], FP8, name=f"ub{h}", tag=f"ub{h}")
                src = ago[:].rearrange("(r k) x -> k r x", k=P)
                for hh in range(2):
                    nc.scalar.dma_start(
                        ub[:, hh * REG:(hh + 1) * REG].rearrange(
                            "k (r x) -> k r x", r=C), src)
                ur = urotp.tile([P, REG], FP8, name=f"ur{h}", tag=f"ur{h}")
                nc.vector.tensor_copy(ur[:], ub[:, bass.ds(sv, REG)])
                return ur

            # ---------- MLP head (bf16): u0^T striped into zpsA/zpsB
            with (
                tc.tile_pool(name="mlpp", bufs=1) as mlpp,
                tc.tile_pool(name="ftp", bufs=2) as ftp,
                tc.tile_pool(name="htp", bufs=2) as htp,
                tc.tile_pool(name="tmpp", bufs=2) as tmpp,
                tc.tile_pool(name="ps1p", bufs=2, space="PSUM") as ps1p,
            ):
                w1_s = mlpp.tile([P, KI * KH * P], BF16, name="w1_s")
                nc.sync.dma_start(w1_s[:], w1_t[:])
                ws_s = mlpp.tile([P, KI * OUT], BF16, name="ws_s")
                nc.sync.dma_start(ws_s[:], ws_t[:])
                w2_s = mlpp.tile([P, KH * OUT], BF16, name="w2_s")
                nc.sync.dma_start(w2_s[:], w2_t[:])
                b1_s = mlpp.tile([P, KH], F32, name="b1_s")
                nc.sync.dma_start(b1_s[:], b1_t[:])

                uT0 = utp.tile([P, NT], F32, name="uT", tag="uT")
                own0 = ownp.tile([P, MT * OUT], FP8, name="own", tag="own")

                fts, hts = {}, {}

                def ph1(nt):
                    ft = ftp.tile([P, KI * NT], BF16, name="ft", tag="ft")
                    nc.sync.dma_start(
                        ft[:].rearrange("k (i r) -> k i r", i=KI),
                        featT[:].rearrange("k (i r) -> k i r", i=KI)[
                            :, :, nt * NT:(nt + 1) * NT])
                    fts[nt] = ft
                    ht = htp.tile([P, KH * NT], BF16, name="ht", tag="ht")
                    hts[nt] = ht
                    for mt in range(KH):
                        ps1 = ps1p.tile([P, NT], F32, name="ps1", tag="ps1")
                        for k in range(KI):
                            nc.tensor.matmul(
                                ps1[:],
                                w1_s[:, (k * KH + mt) * P:(k * KH + mt + 1) * P],
                                ft[:, k * NT:(k + 1) * NT],
                                start=(k == 0), stop=(k == KI - 1))
                        b1_ap = b1_s[:, mt:mt + 1]
                        t_relu = tmpp.tile([P, NT], F32, name="t_relu", tag="tr")
                        nc.scalar.activation(t_relu[:], ps1[:], AF.Relu, bias=b1_ap)
                        t_exp = tmpp.tile([P, NT], F32, name="t_exp", tag="te")
                        nc.scalar.activation(t_exp[:], ps1[:], AF.Exp, bias=b1_ap)
                        t_min = tmpp.tile([P, NT], F32, name="t_min", tag="tm")
                        nc.vector.tensor_scalar(t_min[:], t_exp[:], 1.0, -1.0,
                                                ALU.min, ALU.add)
                        nc.vector.tensor_tensor(
                            ht[:, mt * NT:(mt + 1) * NT],
                            t_min[:], t_relu[:], ALU.add)

                def ph2(nt):
                    g = nt
                    o = zps[g][32 * g:32 * g + OUT, :]
                    for k in range(KI):
                        nc.tensor.matmul(
                            o, ws_s[:, k * OUT:(k + 1) * OUT],
                            fts[nt][:, k * NT:(k + 1) * NT],
                            start=(k == 0), stop=False,
                            tile_position=(0, 32 * g))
                    for k2 in range(KH):
                        nc.tensor.matmul(
                            o, w2_s[:, k2 * OUT:(k2 + 1) * OUT],
                            hts[nt][:, k2 * NT:(k2 + 1) * NT],
                            start=False, stop=(k2 == KH - 1),
                            tile_position=(0, 32 * g))

                ph1(0); ph1(1); ph2(0); ph1(2); ph2(1)
                agoA = tail_compute(-1, 0, uT0, own0)
                ph1(3); ph2(2); ph2(3)
                agoB = tail_compute(-1, 1, uT0, own0)
                state["urA"] = tail_recv(0, agoA)
                state["urB"] = tail_recv(1, agoB)
                state["uT"], state["own"] = uT0, own0

            # ---------- diffusion steps
            with tc.tile_pool(name="slabp", bufs=cfg.SLAB_BUFS) as slabp:
                PAT = [1, 0, 1, 0, 1]  # groups 2,3 issue at 3/5 rate
                for t in range(STEPS):
                    uT_new = utp.tile([P, NT], F32, name="uT", tag="uT")
                    own_new = None if t == STEPS - 1 else ownp.tile(
                        [P, MT * OUT], FP8, name="own", tag="own")
                    slabs = [{}, {}]  # per half-pair streams

                    def get_rhs(p, g):
                        if p < RESP:
                            return res[:, p * R + g * NT: p * R + (g + 1) * NT]
                        hh = 0 if g < 2 else 1
                        q4 = (p - RESP) // 4
                        if q4 not in slabs[hh]:
                            s4 = slabp.tile([P, 4 * HR], FP8, name=f"slab{hh}",
                                            tag=f"slab{hh}")
                            p0 = RESP + q4 * 4
                            nc.sync.dma_start(
                                s4[:].rearrange("k (q m) -> k q m", q=4),
                                lapT[p0 * P:(p0 + 4) * P,
                                     hh * HR:(hh + 1) * HR].rearrange(
                                    "(q k) m -> k q m", k=P))
                            slabs[hh][q4] = s4
                        cq = (p - RESP) % 4
                        off = cq * HR + (g % 2) * NT
                        return slabs[hh][q4][:, off:off + NT]

                    def mm(g, p):
                        w = wslice(p, state["own"], state["urA"], state["urB"])
                        nc.tensor.matmul(
                            zps[g][32 * g:32 * g + OUT, :], w, get_rhs(p, g),
                            start=(p == 0), stop=(p == KC - 1),
                            tile_position=(0, 32 * g))

                    # own positions, all 4 groups at full rate
                    for p in range(MT):
                        for g in range(4):
                            mm(g, p)
                    # staggered foreign sweep
                    pA = pB = MT
                    ib = 0
                    while pA < KC:
                        mm(0, pA); mm(1, pA)
                        pA += 1
                        if PAT[ib % 5] and pB < pA and pB < KC:
                            mm(2, pB); mm(3, pB)
                            pB += 1
                        ib += 1
                    agoA = tail_compute(t, 0, uT_new, own_new)
                    while pB < KC:
                        mm(2, pB); mm(3, pB)
                        pB += 1
                    agoB = tail_compute(t, 1, uT_new, own_new)
                    if t < STEPS - 1:
                        state["urA"] = tail_recv(0, agoA)
                        state["urB"] = tail_recv(1, agoB)
                    state["uT"], state["own"] = uT_new, own_new

    nc.compile()
    return nc


def host_prep(cfg: Cfg, features, laplacian, W1, b1, W2, b2, Ws, bs):
    C, R, MT, KC, OUT = cfg.C, cfg.R, cfg.MT, cfg.KC, cfg.OUT
    KI, KH = cfg.IN_F // P, cfg.HID // P
    HB = MT // 2
    F = np.ascontiguousarray(np.asarray(features, np.float32))
    L = np.asarray(laplacian, np.float32)
    W1 = np.asarray(W1, np.float32)
    b1 = np.asarray(b1, np.float32)
    W2 = np.asarray(W2, np.float32)
    b2 = np.asarray(b2, np.float32)
    Ws = np.asarray(Ws, np.float32)
    bs = np.asarray(bs, np.float32)

    Lq = (L * np.float32(cfg.SIGMA2 * cfg.SCALE)).astype(ml_dtypes.float8_e4m3)

    w1_t = np.ascontiguousarray(
        W1.reshape(KI, P, KH, P).transpose(1, 0, 2, 3).reshape(P, KI * KH * P)
    ).astype(ml_dtypes.bfloat16)
    ws_t = np.ascontiguousarray(
        Ws.reshape(KI, P, OUT).transpose(1, 0, 2).reshape(P, KI * OUT)
    ).astype(ml_dtypes.bfloat16)
    w2_t = np.ascontiguousarray(
        W2.reshape(KH, P, OUT).transpose(1, 0, 2).reshape(P, KH * OUT)
    ).astype(ml_dtypes.bfloat16)
    b1_t = np.ascontiguousarray(b1.reshape(KH, P).T)

    bias = (bs + b2).astype(np.float32)
    biasT = np.zeros((P, 1), np.float32)
    ident = np.zeros((P, OUT), np.float32)
    for g in range(4):
        biasT[32 * g:32 * g + OUT, 0] = bias
        ident[32 * g:32 * g + OUT, :] = np.eye(OUT, dtype=np.float32)

    def perm_for(i):
        # processing position p -> global chunk id
        perm = [MT * i + p for p in range(MT)]
        for half in range(2):
            for q in range(1, C):
                for l in range(HB):
                    perm.append(MT * ((i + q) % C) + half * HB + l)
        return perm

    in_maps = []
    for i in range(C):
        shard = Lq[i * R:(i + 1) * R, :]                   # [R, N]
        Ti = np.ascontiguousarray(shard.T)                 # [N, R] lhsT layout
        Ti_p = np.ascontiguousarray(
            Ti.reshape(KC, P, R)[perm_for(i)].reshape(cfg.N, R))
        Fi = F[i * R:(i + 1) * R, :]
        featT_i = np.ascontiguousarray(
            Fi.T.reshape(KI, P, R).transpose(1, 0, 2).reshape(P, KI * R)
        ).astype(ml_dtypes.bfloat16)
        in_maps.append({
            "lapT": Ti_p,
            "featT": featT_i,
            "w1_t": w1_t,
            "ws_t": ws_t,
            "w2_t": w2_t,
            "b1_t": b1_t,
            "biasT": biasT,
            "ident": ident,
            "rot": np.array([[i * HB * OUT]], np.uint32),
        })
    return in_maps


_NC_CACHE = {}


def _get_nc(cfg: Cfg):
    if cfg not in _NC_CACHE:
        _NC_CACHE[cfg] = build_program(cfg)
    return _NC_CACHE[cfg]


def _install_ntff_hook():
    """Recreate antenv.axon_hooks (absent in this image) so
    run_bass_kernel_spmd(trace=True) can NTFF-profile via libaxon_pjrt."""
    import sys
    import types
    import ctypes
    import contextlib

    if "antenv.axon_hooks" in sys.modules:
        return
    so_path = "/opt/axon/libaxon_pjrt.so"
    lib = ctypes.CDLL(so_path)
    if not hasattr(lib, "axon_start_nrt_profile"):
        return
    lib.axon_start_nrt_profile.argtypes = [
        ctypes.POINTER(ctypes.c_int64), ctypes.c_size_t]
    lib.axon_start_nrt_profile.restype = ctypes.c_int64
    lib.axon_stop_nrt_profile.argtypes = [ctypes.c_char_p]
    lib.axon_stop_nrt_profile.restype = ctypes.c_int64

    @contextlib.contextmanager
    def _hook(output_dir, device_ids):
        import jax
        jax.devices()
        if device_ids:
            ids = (ctypes.c_int64 * len(device_ids))(*device_ids)
            rc = lib.axon_start_nrt_profile(ids, len(device_ids))
        else:
            rc = lib.axon_start_nrt_profile(None, 0)
        if rc != 0:
            raise RuntimeError(f"axon_start_nrt_profile rc={rc}")
        try:
            yield
        finally:
            n = lib.axon_stop_nrt_profile(str(output_dir).encode())
            print(f"profile: {n} file(s) written to {output_dir}")

    mod = types.ModuleType("antenv.axon_hooks")
    mod.get_axon_ntff_profile_hook = lambda: _hook
    mod.set_axon_ntff_profile_hook = lambda h: None
    sys.modules["antenv.axon_hooks"] = mod


def run(inputs, cfg: Cfg = Cfg(), trace: bool = False):
    if trace:
        _install_ntff_hook()
    nc = _get_nc(cfg)
    in_maps = host_prep(cfg, **inputs)
    res = run_bass_kernel_spmd(nc, in_maps, core_ids=list(range(cfg.C)),
                               trace=trace)
    out = np.concatenate([res.results[i]["out_u"] for i in range(cfg.C)], axis=0)
    return out, res


def kernel(**inputs):
    out, _ = run(inputs)
    return out


# revision 12
# speedup vs baseline: 1.5452x; 1.2028x over previous
"""Trainium2 Bass kernel: GNN message passing (iterative Laplacian diffusion).

Problem: u0 = F@Ws + bs + elu(F@W1 + b1)@W2 + b2;  16x: u <- u - 0.1*(L@u)
  F: [16384, 512] fp32, L: [16384, 16384] fp32, output u: [16384, 16] fp32.

v2 strategy (8 NeuronCores, row-parallel SpMM):
  - L row-sharded (2048 rows/core), compressed to fp8e4 (scale sigma2*2^17).
    Each of the 16 steps re-streams the L shard => memory bound; RESP chunks
    (~19 MiB) stay resident in SBUF, the rest stream per step.
  - PE mapping: z^T[j, n] accumulated over 128 k-chunks.  Four col-tiled
    matmul groups (tile_position (0,32g)) run CONCURRENTLY, one per 512-node
    n-tile, each streaming its own L slab slice (4B/part/cycle aggregate,
    2x the DoubleRow rate).  Groups 0,1 share one PSUM bank via a single
    start/stop accumulation chain; groups 2,3 share another.
  - Two half-AllGathers per step, software-pipelined: groups 2,3 issue at
    3/5 the rate of groups 0,1, so half A of z finishes ~9us early; its
    AXPY/transpose/cast/AllGather chain runs while half B still matmuls.
    Each step processes own chunks first, then half-A foreigns, then
    half-B foreigns, so gathers land just in time and the PE never idles
    long enough to drop the HAM clock gate to the cold 1.2GHz state.
  - u is carried TRANSPOSED ([16, nodes] in 4 partition stripes 32g..32g+15
    of a [128,512] tile).  AXPY runs in that layout straight from PSUM;
    PE transposes produce the node-major fp8 copy used as stationary
    weights (and gathered).  Rotation trick: host permutes L chunks so all
    cores run one instruction stream; a DVE register-offset copy per half
    rotates the gathered u into processing order.
"""

import numpy as np
import ml_dtypes
from dataclasses import dataclass

from concourse import bass, bacc, tile
import concourse.mybir as mybir
from concourse.bass_utils import run_bass_kernel_spmd

F32 = mybir.dt.float32
BF16 = mybir.dt.bfloat16
FP8 = mybir.dt.float8e4
U32 = mybir.dt.uint32
P = 128  # partitions


@dataclass(frozen=True)
class Cfg:
    C: int = 8          # cores
    N: int = 16384      # nodes
    IN_F: int = 512     # input features
    HID: int = 256      # hidden dim
    OUT: int = 16       # output features
    STEPS: int = 16
    SIGMA2: float = 0.1
    SCALE: float = 2.0 ** 17   # fp8 encoding scale for (sigma2*L)
    RESP: int = 76      # resident L chunks (of KC) per core
    SLAB_BUFS: int = 3  # streamed half-slab double-buffering

    @property
    def R(self):   # rows per core
        return self.N // self.C

    @property
    def MT(self):  # 128-row blocks per core
        return self.R // P

    @property
    def KC(self):  # total k-chunks
        return self.N // P

    @property
    def NT(self):  # n-tile width (one col-tile group's slice)
        return 512


def build_program(cfg: Cfg):
    C, R, MT, KC, OUT, STEPS = cfg.C, cfg.R, cfg.MT, cfg.KC, cfg.OUT, cfg.STEPS
    IN_F, HID, NT = cfg.IN_F, cfg.HID, cfg.NT
    KI = IN_F // P   # 4 input-feature k-chunks
    KH = HID // P    # 2 hidden k-chunks
    NNT = R // NT    # 4 n-tiles = 4 col-tile groups
    RESP = cfg.RESP
    AXPY_C = -1.0 / cfg.SCALE
    HB = MT // 2          # node blocks per half (8)
    HCOLS = HB * OUT      # own fp8 cols per half (128)
    REG = C * HCOLS       # gathered region cols per half (1024)
    HR = R // 2           # slab columns per half (1024)

    assert RESP % 4 == 0 and RESP >= MT

    nc = bacc.Bacc("TRN2", target_bir_lowering=False, debug=False,
                   enable_asserts=False, num_devices=C)

    lapT = nc.dram_tensor("lapT", [cfg.N, R], FP8, kind="ExternalInput")
    featT = nc.dram_tensor("featT", [P, KI * R], BF16, kind="ExternalInput")
    w1_t = nc.dram_tensor("w1_t", [P, KI * KH * P], BF16, kind="ExternalInput")
    ws_t = nc.dram_tensor("ws_t", [P, KI * OUT], BF16, kind="ExternalInput")
    w2_t = nc.dram_tensor("w2_t", [P, KH * OUT], BF16, kind="ExternalInput")
    b1_t = nc.dram_tensor("b1_t", [P, KH], F32, kind="ExternalInput")
    biasT = nc.dram_tensor("biasT", [P, 1], F32, kind="ExternalInput")
    ident = nc.dram_tensor("ident", [P, OUT], F32, kind="ExternalInput")
    rot = nc.dram_tensor("rot", [1, 1], U32, kind="ExternalInput")
    out_u = nc.dram_tensor("out_u", [R, OUT], F32, kind="ExternalOutput")

    AF = mybir.ActivationFunctionType
    ALU = mybir.AluOpType

    # position p (processing order) -> stationary-weight slice
    # p<16: own block p; 16<=p<72: half-A foreign; p>=72: half-B foreign
    def wslice(p, own, u_rotA, u_rotB):
        if p < MT:
            return own[:, p * OUT:(p + 1) * OUT]
        if p < MT + 7 * HB:
            d = p - MT
            q, l = 1 + d // HB, d % HB
            off = q * HCOLS + l * OUT
            return u_rotA[:, off:off + OUT]
        d = p - (MT + 7 * HB)
        q, l = 1 + d // HB, d % HB
        off = q * HCOLS + l * OUT
        return u_rotB[:, off:off + OUT]

    with tile.TileContext(nc) as tc:
        with (
            tc.tile_pool(name="resp", bufs=1) as resp,
            tc.tile_pool(name="constp", bufs=1) as constp,
            tc.tile_pool(name="utp", bufs=2) as utp,
            tc.tile_pool(name="ownp", bufs=2) as ownp,
            tc.tile_pool(name="ubp", bufs=1) as ubp,
            tc.tile_pool(name="urotp", bufs=2) as urotp,
            tc.tile_pool(name="zps0_p", bufs=1, space="PSUM") as zps0_p,
            tc.tile_pool(name="zps1_p", bufs=1, space="PSUM") as zps1_p,
            tc.tile_pool(name="zps2_p", bufs=1, space="PSUM") as zps2_p,
            tc.tile_pool(name="zps3_p", bufs=1, space="PSUM") as zps3_p,
            tc.tile_pool(name="ptp", bufs=2, space="PSUM") as ptp,
            tc.tile_pool(name="dramp", bufs=2, space="DRAM") as dramp,
        ):
            # ---- rotation register (per-core offset into gathered halves)
            rot_s = constp.tile([1, 1], U32, name="rot_s")
            nc.sync.dma_start(rot_s[:], rot[:])
            reg = nc.alloc_registers("rotreg", engines=[mybir.EngineType.DVE])
            nc.regs_load(reg, rot_s[0:1, 0:1])
            sv = nc.snap(reg, donate=True, min_val=0, max_val=(C - 1) * HCOLS)

            ident_s = constp.tile([P, OUT], F32, name="ident_s")
            nc.sync.dma_start(ident_s[:], ident[:])
            biasT_s = constp.tile([P, 1], F32, name="biasT_s")
            nc.sync.dma_start(biasT_s[:], biasT[:])
            uout = constp.tile([P, MT * OUT], F32, name="uout")

            # ---- resident L chunks, interleaved with streamed ones so HBM
            # demand is spread across the sweep.  Resident positions:
            # [0,16) own, evens in [16,120), [120,128).  Streamed: odds.
            def res_idx(p):
                if p < MT:
                    return p
                if p >= 120:
                    return MT + 52 + (p - 120)
                assert (p - MT) % 2 == 0
                return MT + (p - MT) // 2

            res = resp.tile([P, RESP * R], FP8, name="res")
            nc.sync.dma_start(
                res[:, 0:MT * R].rearrange("k (q m) -> k q m", q=MT),
                lapT[0:MT * P, :].rearrange("(q k) m -> k q m", k=P))
            for b0 in range(0, 52, 13):  # even positions 16..118, 13 at a time
                nc.sync.dma_start(
                    res[:, (MT + b0) * R:(MT + b0 + 13) * R].rearrange(
                        "k (q s m) -> k q s m", q=13, s=1),
                    lapT[(MT + 2 * b0) * P:(MT + 2 * (b0 + 13)) * P,
                         :].rearrange("(q s k) m -> k q s m", s=2, k=P)[
                        :, :, 0:1, :])
            nc.sync.dma_start(
                res[:, (MT + 52) * R:RESP * R].rearrange("k (q m) -> k q m", q=8),
                lapT[120 * P:KC * P, :].rearrange("(q k) m -> k q m", k=P))

            # ---- PSUM tiles: one bank per col-tile group
            zps = [
                zps0_p.tile([P, NT], F32, name="zps0", tag="zps0"),
                zps1_p.tile([P, NT], F32, name="zps1", tag="zps1"),
                zps2_p.tile([P, NT], F32, name="zps2", tag="zps2"),
                zps3_p.tile([P, NT], F32, name="zps3", tag="zps3"),
            ]

            state = {"uT": None, "own": None, "urA": None, "urB": None}

            # ---------- tail compute: half h of step t (t==-1: init/bias-add)
            # axpy -> 8 transposes -> fp8 cast -> agin DMA -> AllGather trigger
            def tail_compute(t, h, uT_new, own_new):
                for g in (0, 1) if h == 0 else (2, 3):
                    lo = 32 * g
                    if t < 0:
                        nc.vector.tensor_scalar_add(
                            uT_new[lo:lo + OUT, :], zps[g][lo:lo + OUT, :],
                            biasT_s[lo:lo + OUT, 0:1])
                    else:
                        nc.vector.scalar_tensor_tensor(
                            uT_new[lo:lo + OUT, :], zps[g][lo:lo + OUT, :],
                            AXPY_C, state["uT"][lo:lo + OUT, :],
                            ALU.mult, ALU.add)
                dst = uout if t == STEPS - 1 else own_new
                for bi in range(HB):
                    b = h * HB + bi
                    g = b // NNT
                    pt = ptp.tile([P, OUT], F32, name="pt", tag="pt")
                    nc.tensor.matmul(
                        pt[:],
                        uT_new[32 * g:32 * g + OUT,
                               (b % NNT) * P:(b % NNT + 1) * P],
                        ident_s[32 * g:32 * g + OUT, :],
                        is_transpose=True, tile_position=(32 * g, 0),
                    )
                    nc.scalar.activation(
                        dst[:, b * OUT:(b + 1) * OUT], pt[:], AF.Copy)
                if t == STEPS - 1:
                    if h == 1:
                        nc.gpsimd.dma_start(
                            out_u[:].rearrange("(m k) j -> k m j", k=P),
                            uout[:].rearrange("k (m j) -> k m j", j=OUT))
                    return None
                agi = dramp.tile([P, HCOLS], FP8, name=f"agi{h}", tag=f"agi{h}")
                ago = dramp.tile([C * P, HCOLS], FP8, name=f"ago{h}",
                                 tag=f"ago{h}", addr_space="Shared")
                nc.scalar.dma_start(agi[:], own_new[:, h * HCOLS:(h + 1) * HCOLS])
                nc.gpsimd.collective_compute(
                    "AllGather", ALU.bypass, replica_groups=[list(range(C))],
                    ins=[agi.opt()], outs=[ago.opt()])
                return ago

            # ---------- tail recv: doubled rearrange-in + rotation copy
            def tail_recv(h, ago):
                ub = ubp.tile([P, 2 * REG], FP8, name=f"ub{h}", tag=f"ub{h}")
                src = ago[:].rearrange("(r k) x -> k r x", k=P)
                for hh in range(2):
                    nc.scalar.dma_start(
                        ub[:, hh * REG:(hh + 1) * REG].rearrange(
                            "k (r x) -> k r x", r=C), src)
                ur = urotp.tile([P, REG], FP8, name=f"ur{h}", tag=f"ur{h}")
                nc.vector.tensor_copy(ur[:], ub[:, bass.ds(sv, REG)])
                return ur

            # ---------- MLP head (bf16): u0^T striped into zpsA/zpsB
            with (
                tc.tile_pool(name="mlpp", bufs=1) as mlpp,
                tc.tile_pool(name="ftp", bufs=2) as ftp,
                tc.tile_pool(name="htp", bufs=2) as htp,
                tc.tile_pool(name="tmpp", bufs=2) as tmpp,
                tc.tile_pool(name="ps1p", bufs=2, space="PSUM") as ps1p,
            ):
                w1_s = mlpp.tile([P, KI * KH * P], BF16, name="w1_s")
                nc.sync.dma_start(w1_s[:], w1_t[:])
                ws_s = mlpp.tile([P, KI * OUT], BF16, name="ws_s")
                nc.sync.dma_start(ws_s[:], ws_t[:])
                w2_s = mlpp.tile([P, KH * OUT], BF16, name="w2_s")
                nc.sync.dma_start(w2_s[:], w2_t[:])
                b1_s = mlpp.tile([P, KH], F32, name="b1_s")
                nc.sync.dma_start(b1_s[:], b1_t[:])

                uT0 = utp.tile([P, NT], F32, name="uT", tag="uT")
                own0 = ownp.tile([P, MT * OUT], FP8, name="own", tag="own")

                fts, hts = {}, {}

                def ph1(nt):
                    ft = ftp.tile([P, KI * NT], BF16, name="ft", tag="ft")
                    nc.sync.dma_start(
                        ft[:].rearrange("k (i r) -> k i r", i=KI),
                        featT[:].rearrange("k (i r) -> k i r", i=KI)[
                            :, :, nt * NT:(nt + 1) * NT])
                    fts[nt] = ft
                    ht = htp.tile([P, KH * NT], BF16, name="ht", tag="ht")
                    hts[nt] = ht
                    for mt in range(KH):
                        ps1 = ps1p.tile([P, NT], F32, name="ps1", tag="ps1")
                        for k in range(KI):
                            nc.tensor.matmul(
                                ps1[:],
                                w1_s[:, (k * KH + mt) * P:(k * KH + mt + 1) * P],
                                ft[:, k * NT:(k + 1) * NT],
                                start=(k == 0), stop=(k == KI - 1))
                        b1_ap = b1_s[:, mt:mt + 1]
                        t_relu = tmpp.tile([P, NT], F32, name="t_relu", tag="tr")
                        nc.scalar.activation(t_relu[:], ps1[:], AF.Relu, bias=b1_ap)
                        t_exp = tmpp.tile([P, NT], F32, name="t_exp", tag="te")
                        nc.scalar.activation(t_exp[:], ps1[:], AF.Exp, bias=b1_ap)
                        t_min = tmpp.tile([P, NT], F32, name="t_min", tag="tm")
                        nc.vector.tensor_scalar(t_min[:], t_exp[:], 1.0, -1.0,
                                                ALU.min, ALU.add)
                        nc.vector.tensor_tensor(
                            ht[:, mt * NT:(mt + 1) * NT],
                            t_min[:], t_relu[:], ALU.add)

                def ph2(nt):
                    g = nt
                    o = zps[g][32 * g:32 * g + OUT, :]
                    for k in range(KI):
                        nc.tensor.matmul(
                            o, ws_s[:, k * OUT:(k + 1) * OUT],
                            fts[nt][:, k * NT:(k + 1) * NT],
                            start=(k == 0), stop=False,
                            tile_position=(0, 32 * g))
                    for k2 in range(KH):
                        nc.tensor.matmul(
                            o, w2_s[:, k2 * OUT:(k2 + 1) * OUT],
                            hts[nt][:, k2 * NT:(k2 + 1) * NT],
                            start=False, stop=(k2 == KH - 1),
                            tile_position=(0, 32 * g))

                ph1(0); ph1(1); ph2(0); ph1(2); ph2(1)
                agoA = tail_compute(-1, 0, uT0, own0)
                ph1(3); ph2(2); ph2(3)
                agoB = tail_compute(-1, 1, uT0, own0)
                state["urA"] = tail_recv(0, agoA)
                state["urB"] = tail_recv(1, agoB)
                state["uT"], state["own"] = uT0, own0

            # ---------- diffusion steps
            with tc.tile_pool(name="slabp", bufs=cfg.SLAB_BUFS) as slabp:
                PAT = [1, 0, 1, 0, 1]  # groups 2,3 issue at 3/5 rate
                for t in range(STEPS):
                    uT_new = utp.tile([P, NT], F32, name="uT", tag="uT")
                    own_new = None if t == STEPS - 1 else ownp.tile(
                        [P, MT * OUT], FP8, name="own", tag="own")
                    slabs = [{}, {}]  # per half-pair streams

                    def get_rhs(p, g):
                        streamed = (MT <= p < 120) and ((p - MT) % 2 == 1)
                        if not streamed:
                            ri = res_idx(p)
                            return res[:, ri * R + g * NT: ri * R + (g + 1) * NT]
                        hh = 0 if g < 2 else 1
                        q4 = (p - MT - 1) // 8
                        if q4 not in slabs[hh]:
                            s4 = slabp.tile([P, 4 * HR], FP8, name=f"slab{hh}",
                                            tag=f"slab{hh}")
                            p0 = MT + 1 + q4 * 8  # first odd position of group
                            nc.sync.dma_start(
                                s4[:].rearrange("k (q s m) -> k q s m",
                                                q=4, s=1),
                                lapT[(p0 - 1) * P:(p0 - 1 + 8) * P,
                                     hh * HR:(hh + 1) * HR].rearrange(
                                    "(q s k) m -> k q s m", s=2, k=P)[
                                    :, :, 1:2, :])
                            slabs[hh][q4] = s4
                        cq = ((p - MT - 1) % 8) // 2
                        off = cq * HR + (g % 2) * NT
                        return slabs[hh][q4][:, off:off + NT]

                    def mm(g, p):
                        w = wslice(p, state["own"], state["urA"], state["urB"])
                        nc.tensor.matmul(
                            zps[g][32 * g:32 * g + OUT, :], w, get_rhs(p, g),
                            start=(p == 0), stop=(p == KC - 1),
                            tile_position=(0, 32 * g))

                    # own positions, all 4 groups at full rate
                    for p in range(MT):
                        for g in range(4):
                            mm(g, p)
                    # staggered foreign sweep
                    pA = pB = MT
                    ib = 0
                    while pA < KC:
                        mm(0, pA); mm(1, pA)
                        pA += 1
                        if PAT[ib % 5] and pB < pA and pB < KC:
                            mm(2, pB); mm(3, pB)
                            pB += 1
                        ib += 1
                    agoA = tail_compute(t, 0, uT_new, own_new)
                    while pB < KC:
                        mm(2, pB); mm(3, pB)
                        pB += 1
                    agoB = tail_compute(t, 1, uT_new, own_new)
                    if t < STEPS - 1:
                        state["urA"] = tail_recv(0, agoA)
                        state["urB"] = tail_recv(1, agoB)
                    state["uT"], state["own"] = uT_new, own_new

    nc.compile()
    return nc


def host_prep(cfg: Cfg, features, laplacian, W1, b1, W2, b2, Ws, bs):
    C, R, MT, KC, OUT = cfg.C, cfg.R, cfg.MT, cfg.KC, cfg.OUT
    KI, KH = cfg.IN_F // P, cfg.HID // P
    HB = MT // 2
    F = np.ascontiguousarray(np.asarray(features, np.float32))
    L = np.asarray(laplacian, np.float32)
    W1 = np.asarray(W1, np.float32)
    b1 = np.asarray(b1, np.float32)
    W2 = np.asarray(W2, np.float32)
    b2 = np.asarray(b2, np.float32)
    Ws = np.asarray(Ws, np.float32)
    bs = np.asarray(bs, np.float32)

    Lq = (L * np.float32(cfg.SIGMA2 * cfg.SCALE)).astype(ml_dtypes.float8_e4m3)

    w1_t = np.ascontiguousarray(
        W1.reshape(KI, P, KH, P).transpose(1, 0, 2, 3).reshape(P, KI * KH * P)
    ).astype(ml_dtypes.bfloat16)
    ws_t = np.ascontiguousarray(
        Ws.reshape(KI, P, OUT).transpose(1, 0, 2).reshape(P, KI * OUT)
    ).astype(ml_dtypes.bfloat16)
    w2_t = np.ascontiguousarray(
        W2.reshape(KH, P, OUT).transpose(1, 0, 2).reshape(P, KH * OUT)
    ).astype(ml_dtypes.bfloat16)
    b1_t = np.ascontiguousarray(b1.reshape(KH, P).T)

    bias = (bs + b2).astype(np.float32)
    biasT = np.zeros((P, 1), np.float32)
    ident = np.zeros((P, OUT), np.float32)
    for g in range(4):
        biasT[32 * g:32 * g + OUT, 0] = bias
        ident[32 * g:32 * g + OUT, :] = np.eye(OUT, dtype=np.float32)

    def perm_for(i):
        # processing position p -> global chunk id
        perm = [MT * i + p for p in range(MT)]
        for half in range(2):
            for q in range(1, C):
                for l in range(HB):
                    perm.append(MT * ((i + q) % C) + half * HB + l)
        return perm

    in_maps = []
    for i in range(C):
        shard = Lq[i * R:(i + 1) * R, :]                   # [R, N]
        Ti = np.ascontiguousarray(shard.T)                 # [N, R] lhsT layout
        Ti_p = np.ascontiguousarray(
            Ti.reshape(KC, P, R)[perm_for(i)].reshape(cfg.N, R))
        Fi = F[i * R:(i + 1) * R, :]
        featT_i = np.ascontiguousarray(
            Fi.T.reshape(KI, P, R).transpose(1, 0, 2).reshape(P, KI * R)
        ).astype(ml_dtypes.bfloat16)
        in_maps.append({
            "lapT": Ti_p,
            "featT": featT_i,
            "w1_t": w1_t,
            "ws_t": ws_t,
            "w2_t": w2_t,
            "b1_t": b1_t,
            "biasT": biasT,
            "ident": ident,
            "rot": np.array([[i * HB * OUT]], np.uint32),
        })
    return in_maps


_NC_CACHE = {}


def _get_nc(cfg: Cfg):
    if cfg not in _NC_CACHE:
        _NC_CACHE[cfg] = build_program(cfg)
    return _NC_CACHE[cfg]


def _install_ntff_hook():
    """Recreate antenv.axon_hooks (absent in this image) so
    run_bass_kernel_spmd(trace=True) can NTFF-profile via libaxon_pjrt."""
    import sys
    import types
    import ctypes
    import contextlib

    if "antenv.axon_hooks" in sys.modules:
        return
    so_path = "/opt/axon/libaxon_pjrt.so"
    lib = ctypes.CDLL(so_path)
    if not hasattr(lib, "axon_start_nrt_profile"):
        return
    lib.axon_start_nrt_profile.argtypes = [
        ctypes.POINTER(ctypes.c_int64), ctypes.c_size_t]
    lib.axon_start_nrt_profile.restype = ctypes.c_int64
    lib.axon_stop_nrt_profile.argtypes = [ctypes.c_char_p]
    lib.axon_stop_nrt_profile.restype = ctypes.c_int64

    @contextlib.contextmanager
    def _hook(output_dir, device_ids):
        import jax
        jax.devices()
        if device_ids:
            ids = (ctypes.c_int64 * len(device_ids))(*device_ids)
            rc = lib.axon_start_nrt_profile(ids, len(device_ids))
        else:
            rc = lib.axon_start_nrt_profile(None, 0)
        if rc != 0:
            raise RuntimeError(f"axon_start_nrt_profile rc={rc}")
        try:
            yield
        finally:
            n = lib.axon_stop_nrt_profile(str(output_dir).encode())
            print(f"profile: {n} file(s) written to {output_dir}")

    mod = types.ModuleType("antenv.axon_hooks")
    mod.get_axon_ntff_profile_hook = lambda: _hook
    mod.set_axon_ntff_profile_hook = lambda h: None
    sys.modules["antenv.axon_hooks"] = mod


def run(inputs, cfg: Cfg = Cfg(), trace: bool = False):
    if trace:
        _install_ntff_hook()
    nc = _get_nc(cfg)
    in_maps = host_prep(cfg, **inputs)
    res = run_bass_kernel_spmd(nc, in_maps, core_ids=list(range(cfg.C)),
                               trace=trace)
    out = np.concatenate([res.results[i]["out_u"] for i in range(cfg.C)], axis=0)
    return out, res


def kernel(**inputs):
    out, _ = run(inputs)
    return out
